# revision 2
# baseline (speedup 1.0000x reference)
"""Trainium2 Bass kernel for nn_LlamaMoDDecoderLayer (MoD decoder layer).

Strategy (8 NeuronCores, tensor-parallel, feature-major layouts):
  - All activations transposed: X^T [feature, token]; every matmul contracts
    over partitions with no activation transposes.
  - Attention: heads sharded 2/core; transposed-scores causal softmax with
    denominator via ones-matmul; per-core head context AllGathered (bf16);
    Wo column-sharded.
  - hs2 = hs + mask_attn*attn computed per-core on its 256 rows (fp32 kept
    for the final residual), AllGathered bf16 for replicated RMSNorm2.
  - MLP: w_gate/w_up column-sharded, w_down row-sharded; bf16 partial
    outputs summed by ReduceScatter so each core ends with its 256 rows.
  - Router argmax masks in exact fp32 (GPSIMD MAC chain + fp32 PE reduce).
  - Matmuls bf16 (host-cast weights), fp32 PSUM accumulation.
"""

import numpy as np
import ml_dtypes

import concourse.bass as bass
import concourse.bacc as bacc
import concourse.mybir as mybir
import concourse.tile as tile
from concourse.alu_op_type import AluOpType
from concourse.bass_utils import run_bass_kernel_spmd

F32 = mybir.dt.float32
BF16 = mybir.dt.bfloat16
AF = mybir.ActivationFunctionType

S, D, H, Dh, F = 2048, 2048, 16, 128, 8192
NC = 8
HPC = H // NC            # heads per core (2)
DCC = D // NC            # output cols per core (256)
FPC = F // NC            # mlp hidden per core (1024)
NDT = D // 128           # 16 d-tiles
NFT = FPC // 128         # 8 local f-tiles
NSC = S // 512           # 4 s-chunks of 512
EPS = 1e-5
THETA = 10000.0

_CACHE = {}

def _build_program():
    nc = bacc.Bacc("TRN2", target_bir_lowering=False, debug=False,
                   num_devices=NC)
    rg = [list(range(NC))]

    d_hsT = nc.dram_tensor("hsT", [D, S], F32, kind="ExternalInput")
    d_hres = nc.dram_tensor("hres", [DCC, S], F32, kind="ExternalInput")
    d_wq = nc.dram_tensor("wq", [D, DCC], BF16, kind="ExternalInput")
    d_wk = nc.dram_tensor("wk", [D, DCC], BF16, kind="ExternalInput")
    d_wv = nc.dram_tensor("wv", [D, DCC], BF16, kind="ExternalInput")
    d_wo = nc.dram_tensor("wo", [D, DCC], BF16, kind="ExternalInput")
    d_wg = nc.dram_tensor("wg", [D, FPC], BF16, kind="ExternalInput")
    d_wu = nc.dram_tensor("wu", [D, FPC], BF16, kind="ExternalInput")
    d_wd = nc.dram_tensor("wd", [FPC, D], BF16, kind="ExternalInput")
    d_qcos = nc.dram_tensor("qcos", [Dh, S], BF16, kind="ExternalInput")
    d_qsin = nc.dram_tensor("qsin", [Dh, S], BF16, kind="ExternalInput")
    d_kcos = nc.dram_tensor("kcos", [Dh, S], BF16, kind="ExternalInput")
    d_ksin = nc.dram_tensor("ksin", [Dh, S], BF16, kind="ExternalInput")
    d_tri = nc.dram_tensor("tri", [128, 4 * 512], BF16, kind="ExternalInput")
    d_rd = nc.dram_tensor("rd", [128, 2 * NDT], F32, kind="ExternalInput")
    d_thr = nc.dram_tensor("thr", [1, 2], F32, kind="ExternalInput")
    d_out = nc.dram_tensor("out", [DCC, S], F32, kind="ExternalOutput")
    d_dbgq = nc.dram_tensor("dbgq", [DCC, S], BF16, kind="ExternalOutput")
    d_dbgc = nc.dram_tensor("dbgc", [DCC, S], BF16, kind="ExternalOutput")
    d_dbgh = nc.dram_tensor("dbgh", [DCC, S], BF16, kind="ExternalOutput")

    cc1_in = nc.dram_tensor("cc1_in", [DCC, S], BF16)
    cc1_out = nc.dram_tensor("cc1_out", [D, S], BF16, addr_space="Shared")
    cc2_in = nc.dram_tensor("cc2_in", [DCC, S], BF16)
    cc2_out = nc.dram_tensor("cc2_out", [D, S], BF16, addr_space="Shared")
    cc3_in = nc.dram_tensor("cc3_in", [D, S], BF16)
    cc3_out = nc.dram_tensor("cc3_out", [DCC, S], BF16)

    hsT_t = d_hsT.ap().rearrange("(a p) s -> p a s", p=128)
    hres_t = d_hres.ap().rearrange("(a p) s -> p a s", p=128)
    wq_t = d_wq.ap().rearrange("(a p) m -> p a m", p=128)
    wk_t = d_wk.ap().rearrange("(a p) m -> p a m", p=128)
    wv_t = d_wv.ap().rearrange("(a p) m -> p a m", p=128)
    wo_t = d_wo.ap().rearrange("(a p) m -> p a m", p=128)
    wg_t = d_wg.ap().rearrange("(a p) m -> p a m", p=128)
    wu_t = d_wu.ap().rearrange("(a p) m -> p a m", p=128)
    wd_t = d_wd.ap().rearrange("(a p) m -> p a m", p=128)
    cc1i_t = cc1_in.ap().rearrange("(a p) s -> p a s", p=128)
    cc2i_t = cc2_in.ap().rearrange("(a p) s -> p a s", p=128)
    cc3i_t = cc3_in.ap().rearrange("(a p) s -> p a s", p=128)
    cc1o_t = cc1_out.ap().rearrange("(a p) s -> p a s", p=128)
    cc2o_t = cc2_out.ap().rearrange("(a p) s -> p a s", p=128)
    cc3o_t = cc3_out.ap().rearrange("(a p) s -> p a s", p=128)
    out_t = d_out.ap().rearrange("(a p) s -> p a s", p=128)
    dbgq_t = d_dbgq.ap().rearrange("(a p) s -> p a s", p=128)
    dbgc_t = d_dbgc.ap().rearrange("(a p) s -> p a s", p=128)
    dbgh_t = d_dbgh.ap().rearrange("(a p) s -> p a s", p=128)

    with tile.TileContext(nc) as tc:
        with (
            tc.tile_pool(name="const", bufs=1) as cst,
            tc.tile_pool(name="masks", bufs=1) as mkp,
            tc.tile_pool(name="psum", bufs=2, space="PSUM") as psp,
        ):
            ones_b = cst.tile([128, 1], BF16)
            nc.gpsimd.memset(ones_b[:], 1.0)
            ones_r = cst.tile([1, 128], F32)
            nc.gpsimd.memset(ones_r[:], 1.0)
            ones_f = cst.tile([128, 1], F32)
            nc.gpsimd.memset(ones_f[:], 1.0)
            eps1 = cst.tile([1, 1], F32)
            nc.gpsimd.memset(eps1[:], EPS)
            rd = cst.tile([128, 2 * NDT], F32, name="rd")
            nc.sync.dma_start(rd[:], d_rd.ap())
            thr = cst.tile([1, 2], F32, name="thr")
            nc.sync.dma_start(thr[:], d_thr.ap())
            ma_b = mkp.tile([128, S], F32, name="ma_b")
            mm_b = mkp.tile([128, S], F32, name="mm_b")

            with (
                tc.tile_pool(name="attnconst", bufs=1) as acst,
                tc.tile_pool(name="xn", bufs=1) as xnp,
            ):
                qcos = acst.tile([128, S], BF16, name="qcos")
                qsin = acst.tile([128, S], BF16, name="qsin")
                kcos = acst.tile([128, S], BF16, name="kcos")
                ksin = acst.tile([128, S], BF16, name="ksin")
                nc.sync.dma_start(qcos[:], d_qcos.ap())
                nc.sync.dma_start(qsin[:], d_qsin.ap())
                nc.sync.dma_start(kcos[:], d_kcos.ap())
                nc.sync.dma_start(ksin[:], d_ksin.ap())
                tri = acst.tile([128, 4, 512], BF16, name="tri")
                nc.sync.dma_start(
                    tri[:], d_tri.ap().rearrange("p (a m) -> p a m", m=512))
                xnT = xnp.tile([128, NDT, S], BF16, name="xnT")

                # ---- phase 1: stream hsT twice; routers; norm1; xnT ----
                with tc.tile_pool(name="ph1", bufs=1) as p1:
                    dacc_a = p1.tile([128, S], F32, name="dacc_a")
                    dacc_m = p1.tile([128, S], F32, name="dacc_m")
                    acc = p1.tile([128, S], F32, name="acc")
                    r1b = p1.tile([128, S], F32, name="r1b")
                    r1row = p1.tile([1, S], F32, name="r1row")
                    for a in range(NDT):
                        ht = p1.tile([128, S], F32, tag="hst", bufs=3)
                        nc.sync.dma_start(ht[:], hsT_t[:, a, :])
                        sqt = p1.tile([128, S], BF16, tag="sq", bufs=3)
                        nc.scalar.activation(sqt[:], ht[:], AF.Square)
                        if a == 0:
                            nc.vector.tensor_copy(acc[:], sqt[:])
                        else:
                            nc.vector.tensor_tensor(acc[:], acc[:], sqt[:],
                                                    op=AluOpType.add)
                        if a == 0:
                            nc.vector.tensor_scalar(
                                dacc_a[:], ht[:], rd[:, 0:1], None,
                                op0=AluOpType.mult)
                            nc.vector.tensor_scalar(
                                dacc_m[:], ht[:], rd[:, NDT:NDT + 1], None,
                                op0=AluOpType.mult)
                        else:
                            nc.vector.scalar_tensor_tensor(
                                dacc_a[:], ht[:], rd[:, a:a + 1], dacc_a[:],
                                op0=AluOpType.mult, op1=AluOpType.add)
                            nc.vector.scalar_tensor_tensor(
                                dacc_m[:], ht[:], rd[:, NDT + a:NDT + a + 1],
                                dacc_m[:], op0=AluOpType.mult,
                                op1=AluOpType.add)
                    for sc in range(NSC):
                        rp = psp.tile([1, 512], F32, tag="rowps")
                        nc.tensor.matmul(rp[:], ones_f[:],
                                         acc[:, bass.ts(sc, 512)])
                        nc.scalar.activation(r1row[:, bass.ts(sc, 512)], rp[:],
                                             AF.Sqrt, bias=eps1[:],
                                             scale=1.0 / D)
                        nc.vector.reciprocal(r1row[:, bass.ts(sc, 512)],
                                             r1row[:, bass.ts(sc, 512)])
                        bcp = psp.tile([128, 512], F32, tag="mmps")
                        nc.tensor.matmul(bcp[:], ones_r[:],
                                         r1row[:, bass.ts(sc, 512)])
                        nc.scalar.copy(r1b[:, bass.ts(sc, 512)], bcp[:])
                    for a in range(NDT):
                        ht2 = p1.tile([128, S], F32, tag="hst2", bufs=2)
                        nc.sync.dma_start(ht2[:], hsT_t[:, a, :])
                        nc.vector.tensor_tensor(xnT[:, a, :], ht2[:], r1b[:],
                                                op=AluOpType.mult)
                    for dacc, ti, mb in (
                        (dacc_a, 0, ma_b),
                        (dacc_m, 1, mm_b),
                    ):
                        for sc in range(NSC):
                            dps = psp.tile([1, 512], F32, tag="rowps")
                            nc.tensor.matmul(dps[:], ones_f[:],
                                             dacc[:, bass.ts(sc, 512)])
                            mrow = p1.tile([1, 512], F32, tag="mrow", bufs=2)
                            nc.vector.tensor_scalar(
                                mrow[:], dps[:],
                                thr[:, ti:ti + 1], None, op0=AluOpType.is_le)
                            nc.gpsimd.partition_broadcast(
                                mb[:, bass.ts(sc, 512)], mrow[:])

                # ---- phase 2: QKV + rope; phase 3: attention ----
                with tc.tile_pool(name="qkv", bufs=1) as qkp:
                    wq = qkp.tile([128, NDT, DCC], BF16, name="wq")
                    wk = qkp.tile([128, NDT, DCC], BF16, name="wk")
                    wv = qkp.tile([128, NDT, DCC], BF16, name="wv")
                    nc.sync.dma_start(wq[:], wq_t)
                    nc.sync.dma_start(wk[:], wk_t)
                    nc.sync.dma_start(wv[:], wv_t)
                    q_sb = qkp.tile([128, HPC, S], BF16, name="q_sb")
                    k_sb = qkp.tile([128, HPC, S], BF16, name="k_sb")
                    qs_sb = qkp.tile([128, HPC, S], BF16, name="qs_sb")
                    ks_sb = qkp.tile([128, HPC, S], BF16, name="ks_sb")
                    for w_sb, t_sb in ((wq, q_sb), (wk, k_sb)):
                        for mc in range(HPC):
                            for sc in range(NSC):
                                ps = psp.tile([128, 512], F32, tag="mmps")
                                for a in range(NDT):
                                    nc.tensor.matmul(
                                        ps[:], w_sb[:, a, bass.ts(mc, 128)],
                                        xnT[:, a, bass.ts(sc, 512)],
                                        start=(a == 0), stop=(a == NDT - 1))
                                nc.scalar.copy(t_sb[:, mc, bass.ts(sc, 512)],
                                               ps[:])
                    for mc in range(HPC):
                        nc.sync.dma_start(dbgq_t[:, mc, :], q_sb[:, mc, :])
                    for src, dst in ((q_sb, qs_sb), (k_sb, ks_sb)):
                        for mc in range(HPC):
                            nc.sync.dma_start(dst[0:64, mc, :],
                                              src[64:128, mc, :])
                            nc.sync.dma_start(dst[64:128, mc, :],
                                              src[0:64, mc, :])
                    qr = qkp.tile([128, HPC, S], BF16, name="qr")
                    kr = qkp.tile([128, HPC, S], BF16, name="kr")
                    for mc in range(HPC):
                        tq = qkp.tile([128, S], BF16, tag="ropetmp", bufs=2)
                        nc.vector.tensor_tensor(tq[:], qs_sb[:, mc, :],
                                                qsin[:], op=AluOpType.mult)
                        nc.vector.tensor_tensor(qr[:, mc, :], q_sb[:, mc, :],
                                                qcos[:], op=AluOpType.mult)
                        nc.vector.tensor_tensor(qr[:, mc, :], qr[:, mc, :],
                                                tq[:], op=AluOpType.add)
                        tk = qkp.tile([128, S], BF16, tag="ropetmp", bufs=2)
                        nc.vector.tensor_tensor(tk[:], ks_sb[:, mc, :],
                                                ksin[:], op=AluOpType.mult)
                        nc.vector.tensor_tensor(kr[:, mc, :], k_sb[:, mc, :],
                                                kcos[:], op=AluOpType.mult)
                        nc.vector.tensor_tensor(kr[:, mc, :], kr[:, mc, :],
                                                tk[:], op=AluOpType.add)
                    v_sb = qkp.tile([128, NDT, DCC], BF16, name="v_sb")
                    for mc in range(NDT):
                        ps = psp.tile([128, DCC], F32, tag="mmps")
                        for a in range(NDT):
                            nc.tensor.matmul(ps[:],
                                             xnT[:, a, bass.ts(mc, 128)],
                                             wv[:, a, :],
                                             start=(a == 0),
                                             stop=(a == NDT - 1))
                        nc.scalar.copy(v_sb[:, mc, :], ps[:])

                    ctxT = qkp.tile([128, HPC, S], BF16, name="ctxT")
                    for h in range(HPC):
                        for qc in range(NSC):
                            nkt = 4 * (qc + 1)
                            cps = psp.tile([128, 512], F32, tag="ctxps",
                                           bufs=1)
                            dps = psp.tile([1, 512], F32, tag="rowps")
                            for kt in range(nkt):
                                sps = psp.tile([128, 512], F32, tag="stps")
                                nc.tensor.matmul(sps[:],
                                                 kr[:, h, bass.ts(kt, 128)],
                                                 qr[:, h, bass.ts(qc, 512)])
                                est = qkp.tile([128, 512], BF16, tag="est",
                                               bufs=3)
                                nc.scalar.activation(est[:], sps[:], AF.Exp)
                                if kt // 4 == qc:
                                    nc.vector.tensor_tensor(
                                        est[:], est[:], tri[:, kt % 4, :],
                                        op=AluOpType.mult)
                                nc.tensor.matmul(cps[:],
                                                 v_sb[:, kt, bass.ts(h, 128)],
                                                 est[:], start=(kt == 0),
                                                 stop=(kt == nkt - 1))
                                nc.tensor.matmul(dps[:], ones_b[:], est[:],
                                                 start=(kt == 0),
                                                 stop=(kt == nkt - 1))
                            rrow = qkp.tile([1, 512], F32, tag="rrow", bufs=1)
                            nc.vector.reciprocal(rrow[:], dps[:])
                            rb = qkp.tile([128, 512], F32, tag="rb", bufs=2)
                            nc.gpsimd.partition_broadcast(rb[:], rrow[:])
                            nc.vector.tensor_tensor(
                                ctxT[:, h, bass.ts(qc, 512)], cps[:], rb[:],
                                op=AluOpType.mult)
                    for mc in range(HPC):
                        nc.sync.dma_start(cc1i_t[:, mc, :], ctxT[:, mc, :])
                        nc.sync.dma_start(dbgc_t[:, mc, :], ctxT[:, mc, :])

            # ---- phase 4: AG ctx + Wo proj + hs2 ----
            nc.gpsimd.collective_compute(
                "AllGather", AluOpType.bypass, replica_groups=rg,
                ins=[cc1_in.ap()], outs=[cc1_out.ap()])
            with tc.tile_pool(name="p46", bufs=1) as p46:
                hres = p46.tile([128, 2, S], F32, name="hres")
                nc.sync.dma_start(hres[:], hres_t)
                hs2f = p46.tile([128, 2, S], F32, name="hs2f")
                with tc.tile_pool(name="wo_ph", bufs=1) as wop:
                    ctxg = wop.tile([128, NDT, S], BF16, name="ctxg")
                    for a in range(NDT):
                        nc.sync.dma_start(ctxg[:, a, :], cc1o_t[:, a, :])
                    wo = wop.tile([128, NDT, DCC], BF16, name="wo")
                    nc.sync.dma_start(wo[:], wo_t)
                    hs2b = wop.tile([128, 2, S], BF16, name="hs2b")
                    for mc in range(HPC):
                        for sc in range(NSC):
                            ps = psp.tile([128, 512], F32, tag="mmps")
                            for a in range(NDT):
                                nc.tensor.matmul(
                                    ps[:], wo[:, a, bass.ts(mc, 128)],
                                    ctxg[:, a, bass.ts(sc, 512)],
                                    start=(a == 0), stop=(a == NDT - 1))
                            t = wop.tile([128, 512], F32, tag="wot", bufs=2)
                            nc.vector.tensor_tensor(
                                t[:], ps[:], ma_b[:, bass.ts(sc, 512)],
                                op=AluOpType.mult)
                            nc.vector.tensor_tensor(
                                hs2f[:, mc, bass.ts(sc, 512)], t[:],
                                hres[:, mc, bass.ts(sc, 512)],
                                op=AluOpType.add)
                            nc.scalar.copy(hs2b[:, mc, bass.ts(sc, 512)],
                                           hs2f[:, mc, bass.ts(sc, 512)])
                    for mc in range(HPC):
                        nc.sync.dma_start(cc2i_t[:, mc, :], hs2b[:, mc, :])
                        nc.sync.dma_start(dbgh_t[:, mc, :], hs2b[:, mc, :])
                nc.gpsimd.collective_compute(
                    "AllGather", AluOpType.bypass, replica_groups=rg,
                    ins=[cc2_in.ap()], outs=[cc2_out.ap()])

                # ---- phase 5: norm2 + MLP ----
                with tc.tile_pool(name="mlp", bufs=1) as mlp:
                    hs2g = mlp.tile([128, NDT, S], BF16, name="hs2g")
                    for a in range(NDT):
                        nc.sync.dma_start(hs2g[:, a, :], cc2o_t[:, a, :])
                    with tc.tile_pool(name="r2p", bufs=1) as r2p:
                        r2row = r2p.tile([1, S], F32, name="r2row")
                        r2b = r2p.tile([128, S], F32, name="r2b")
                        for sc in range(NSC):
                            ssp = psp.tile([1, 512], F32, tag="rowps")
                            for a in range(NDT):
                                sqt = r2p.tile([128, 512], BF16, tag="sq2",
                                               bufs=3)
                                nc.scalar.activation(
                                    sqt[:], hs2g[:, a, bass.ts(sc, 512)],
                                    AF.Square)
                                nc.tensor.matmul(ssp[:], ones_b[:], sqt[:],
                                                 start=(a == 0),
                                                 stop=(a == NDT - 1))
                            nc.scalar.activation(r2row[:, bass.ts(sc, 512)],
                                                 ssp[:], AF.Sqrt,
                                                 bias=eps1[:], scale=1.0 / D)
                            nc.vector.reciprocal(r2row[:, bass.ts(sc, 512)],
                                                 r2row[:, bass.ts(sc, 512)])
                            bcp = psp.tile([128, 512], F32, tag="mmps")
                            nc.tensor.matmul(bcp[:], ones_r[:],
                                             r2row[:, bass.ts(sc, 512)])
                            nc.scalar.copy(r2b[:, bass.ts(sc, 512)], bcp[:])
                        for a in range(NDT):
                            nc.vector.tensor_tensor(
                                hs2g[:, a, :], hs2g[:, a, :], r2b[:],
                                op=AluOpType.mult)
                    xn2 = hs2g  # normalized in place
                    hT = mlp.tile([128, NFT, S], BF16, name="hT")
                    with tc.tile_pool(name="wstream", bufs=3) as wsp:
                        for fc in range(NFT):
                            wgc = wsp.tile([128, NDT, 128], BF16, tag="wgc")
                            nc.sync.dma_start(wgc[:],
                                              wg_t[:, :, bass.ts(fc, 128)])
                            sg = wsp.tile([128, S], BF16, tag="sg", bufs=2)
                            for sc in range(NSC):
                                ps = psp.tile([128, 512], F32, tag="mmps")
                                for a in range(NDT):
                                    nc.tensor.matmul(
                                        ps[:], wgc[:, a, :],
                                        xn2[:, a, bass.ts(sc, 512)],
                                        start=(a == 0), stop=(a == NDT - 1))
                                nc.scalar.activation(sg[:, bass.ts(sc, 512)],
                                                     ps[:], AF.Silu)
                            wuc = wsp.tile([128, NDT, 128], BF16, tag="wuc")
                            nc.sync.dma_start(wuc[:],
                                              wu_t[:, :, bass.ts(fc, 128)])
                            for sc in range(NSC):
                                ps = psp.tile([128, 512], F32, tag="mmps")
                                for a in range(NDT):
                                    nc.tensor.matmul(
                                        ps[:], wuc[:, a, :],
                                        xn2[:, a, bass.ts(sc, 512)],
                                        start=(a == 0), stop=(a == NDT - 1))
                                nc.vector.tensor_tensor(
                                    hT[:, fc, bass.ts(sc, 512)], ps[:],
                                    sg[:, bass.ts(sc, 512)],
                                    op=AluOpType.mult)
                        for mc in range(NDT):
                            wdc = wsp.tile([128, NFT, 128], BF16, tag="wdc")
                            nc.sync.dma_start(wdc[:],
                                              wd_t[:, :, bass.ts(mc, 128)])
                            for sc in range(NSC):
                                ps = psp.tile([128, 512], F32, tag="mmps")
                                for a in range(NFT):
                                    nc.tensor.matmul(
                                        ps[:], wdc[:, a, :],
                                        hT[:, a, bass.ts(sc, 512)],
                                        start=(a == 0), stop=(a == NFT - 1))
                                stg = wsp.tile([128, 512], BF16, tag="stg",
                                               bufs=3)
                                nc.scalar.copy(stg[:], ps[:])
                                nc.sync.dma_start(
                                    cc3i_t[:, mc, bass.ts(sc, 512)], stg[:])
                nc.gpsimd.collective_compute(
                    "ReduceScatter", AluOpType.add, replica_groups=rg,
                    ins=[cc3_in.ap()], outs=[cc3_out.ap()])

                # ---- phase 6: final residual ----
                with tc.tile_pool(name="fin", bufs=1) as fin:
                    rs = fin.tile([128, 2, S], BF16, name="rs")
                    for mc in range(HPC):
                        nc.sync.dma_start(rs[:, mc, :], cc3o_t[:, mc, :])
                    outt = fin.tile([128, 2, S], F32, name="outt")
                    for mc in range(HPC):
                        t2 = fin.tile([128, S], F32, tag="fint", bufs=2)
                        nc.vector.tensor_tensor(t2[:], rs[:, mc, :], mm_b[:],
                                                op=AluOpType.mult)
                        nc.vector.tensor_tensor(outt[:, mc, :], t2[:],
                                                hs2f[:, mc, :],
                                                op=AluOpType.add)
                        nc.sync.dma_start(out_t[:, mc, :], outt[:, mc, :])

    nc.compile()
    return nc

def _rope_tables():
    pos = np.arange(S, dtype=np.float32)
    inv = 1.0 / (THETA ** (np.arange(0, Dh, 2, dtype=np.float32) / Dh))
    ang = pos[:, None] * inv[None, :]
    emb = np.concatenate([ang, ang], axis=-1)          # [S, Dh]
    cosT = np.cos(emb).T.astype(np.float32).copy()     # [Dh, S]
    ssinT = np.sin(emb).T.astype(np.float32).copy()
    ssinT[:64] = -ssinT[:64]
    return cosT, ssinT


def _tri_masks():
    # [128, 4, 512] for the diagonal 512-q-chunk, k-tile offset i in chunk:
    # col j: 0 if j < 128i; causal tri inside diag block; 1 past it.
    m = np.zeros((128, 4, 512), np.float32)
    for i in range(4):
        j = np.arange(512)[None, :]
        p = np.arange(128)[:, None]
        m[:, i, :] = ((j - 128 * i) >= p).astype(np.float32)
        m[:, i, : 128 * i] = 0.0
        m[:, i, 128 * (i + 1):] = 1.0
    return m.reshape(128, 4 * 512)


def kernel(**inputs):
    bf = ml_dtypes.bfloat16
    hs = np.ascontiguousarray(np.asarray(inputs["hidden_states"],
                                         np.float32)[0])
    ln1 = np.asarray(inputs["ln1_w"], np.float32)
    ln2 = np.asarray(inputs["ln2_w"], np.float32)
    Wq = np.asarray(inputs["Wq"], np.float32) * ln1[:, None]
    Wk = np.asarray(inputs["Wk"], np.float32) * ln1[:, None]
    Wv = np.asarray(inputs["Wv"], np.float32) * ln1[:, None]
    Wo = np.asarray(inputs["Wo"], np.float32)
    wg = np.asarray(inputs["w_gate"], np.float32) * ln2[:, None]
    wu = np.asarray(inputs["w_up"], np.float32) * ln2[:, None]
    wd = np.asarray(inputs["w_down"], np.float32)
    raw = np.asarray(inputs["router_attn_w"], np.float32)
    rab = np.asarray(inputs["router_attn_b"], np.float32)
    rmw = np.asarray(inputs["router_mlp_w"], np.float32)
    rmb = np.asarray(inputs["router_mlp_b"], np.float32)

    hsT = np.ascontiguousarray(hs.T)                   # [D, S]
    cosT, ssinT = _rope_tables()
    sc = np.float32(1.0 / np.sqrt(Dh))
    rd_a = (raw[:, 1] - raw[:, 0]).astype(np.float32)
    rd_m = (rmw[:, 1] - rmw[:, 0]).astype(np.float32)
    rd = np.concatenate([rd_a.reshape(NDT, 128).T, rd_m.reshape(NDT, 128).T],
                        axis=1).astype(np.float32)     # [128, 32]
    thr = np.array([[-(rab[1] - rab[0]), -(rmb[1] - rmb[0])]], np.float32)
    tri = np.ascontiguousarray(_tri_masks().astype(bf))
    qcos = np.ascontiguousarray(cosT.astype(bf))
    qsin = np.ascontiguousarray(ssinT.astype(bf))
    kcos = np.ascontiguousarray((cosT * sc).astype(bf))
    ksin = np.ascontiguousarray((ssinT * sc).astype(bf))

    if "nc" not in _CACHE:
        _CACHE["nc"] = _build_program()
    nc = _CACHE["nc"]

    in_maps = []
    for c in range(NC):
        dsl = slice(c * DCC, (c + 1) * DCC)
        fsl = slice(c * FPC, (c + 1) * FPC)
        in_maps.append({
            "hsT": hsT,
            "hres": np.ascontiguousarray(hsT[dsl]),
            "wq": np.ascontiguousarray(Wq[:, dsl].astype(bf)),
            "wk": np.ascontiguousarray(Wk[:, dsl].astype(bf)),
            "wv": np.ascontiguousarray(Wv[:, dsl].astype(bf)),
            "wo": np.ascontiguousarray(Wo[:, dsl].astype(bf)),
            "wg": np.ascontiguousarray(wg[:, fsl].astype(bf)),
            "wu": np.ascontiguousarray(wu[:, fsl].astype(bf)),
            "wd": np.ascontiguousarray(wd[fsl].astype(bf)),
            "qcos": qcos, "qsin": qsin, "kcos": kcos, "ksin": ksin,
            "tri": tri, "rd": rd, "thr": thr,
        })
    _CACHE["in_maps"] = in_maps
    res = run_bass_kernel_spmd(nc, in_maps, core_ids=list(range(NC)),
                               trace=bool(globals().get("_TRACE")))
    _CACHE["res"] = res
    outT = np.concatenate([res.results[c]["out"] for c in range(NC)], axis=0)
    return np.ascontiguousarray(outT.T)[None]


if __name__ == "__main__":
    import reference
    inputs = reference.setup_inputs()
    out = kernel(**inputs)
    print(out.shape, out.dtype)

